# revision 5
# baseline (speedup 1.0000x reference)
"""AdaMHF fused kernel for 8 TRN2 NeuronCores.

Strategy: pure data-parallel over batch (B=256 -> 32 per core), all params
replicated. Inputs + weights converted to bf16 on host (halves DMA; fp32
PSUM accumulation keeps precision). Token reductions done on transposed
[D, NTok] tiles: max on DVE (reduce_max), mean on GpSimd (pool_avg). The
MLP stack runs in b-major layout [32, D] with PE transposes producing bf16
lhsT tiles; biases are added via K=1 ones-matmuls into the same PSUM
accumulation group. Weights stream through SBUF as one-use [128, <=512]
blocks issued on the ACT queue (inputs use the SP queue).
"""

import sys
import numpy as np

sys.path.insert(0, "/opt/trn_rl_repo")

import ml_dtypes  # noqa: E402

import concourse.mybir as mybir  # noqa: E402
import concourse.tile as tile  # noqa: E402
from concourse import bacc  # noqa: E402
from concourse.bass_utils import run_bass_kernel_spmd  # noqa: E402
from concourse.masks import make_identity  # noqa: E402

AF = mybir.ActivationFunctionType
ALU = mybir.AluOpType
BF16 = mybir.dt.bfloat16
F32 = mybir.dt.float32

NCORES = 8
B, BL = 256, 32
D, DH, E, R, O, C = 1024, 4096, 4, 16, 256, 4
NP_, NG = 512, 256
KC = D // 128  # 8 k-chunks of the feature dim

_BUILT = {}


def _transpose_to_lhsT(nc, psT, lhsT_pool, ident, src, width, tag):
    """src: f32 sbuf [32, width] -> bf16 lhsT tile [128, width//128, 32]."""
    nch = width // 128
    dst = lhsT_pool.tile([128, nch, 32], BF16, tag=tag, name=tag)
    for c in range(nch):
        pt = psT.tile([128, 32], F32, tag="pT")
        nc.tensor.transpose(pt[:, :], src[:, c * 128:(c + 1) * 128],
                            ident[0:32, 0:32])
        nc.scalar.activation(dst[:, c, :], pt[:, :], AF.Copy)
    return dst


def _build_kernel():
    nc = bacc.Bacc("TRN2", target_bir_lowering=False, debug=False,
                   enable_asserts=False, num_devices=NCORES)

    xp = nc.dram_tensor("xp", [BL, NP_, D], BF16, kind="ExternalInput")
    xo = nc.dram_tensor("xo", [BL, NG, D], BF16, kind="ExternalInput")

    def w(name, shape, dt=BF16):
        return nc.dram_tensor(name, shape, dt, kind="ExternalInput")

    atsa = {}
    for m in ("p", "g"):
        atsa[m] = {
            "w1": w(f"a{m}_w1", [D, D]), "b1": w(f"a{m}_b1", [1, D]),
            "wc": w(f"a{m}_wc", [2 * D, D]), "bc": w(f"a{m}_bc", [1, D]),
            "w2": w(f"a{m}_w2", [D, D]), "b2": w(f"a{m}_b2", [1, D]),
        }
    preeg = {}
    for m in ("p", "g"):
        preeg[m] = {
            "wf1": w(f"p{m}_wf1", [D, DH]), "bf1": w(f"p{m}_bf1", [1, DH]),
            "wf2": w(f"p{m}_wf2", [DH, D]), "bf2": w(f"p{m}_bf2", [1, D]),
            "ew1": w(f"p{m}_ew1", [E, D, D]), "eb1": w(f"p{m}_eb1", [E, D]),
            "ew2": w(f"p{m}_ew2", [E, D, D]), "eb2": w(f"p{m}_eb2", [E, D]),
        }
    fa = w("lmf_fa", [R, D + 1, O])
    fv = w("lmf_fv", [R, D + 1, O])
    fwb = w("lmf_fwb", [BL, R], F32)     # fw broadcast along batch (host)
    beff = w("lmf_beff", [BL, C], F32)   # fb @ cls_w + cls_b, broadcast (host)
    cls_w = w("cls_w", [O, C])

    out_ext = nc.dram_tensor("out", [BL, C], F32, kind="ExternalOutput")

    from contextlib import ExitStack
    with tile.TileContext(nc) as tc, ExitStack() as ctx:
        consts = ctx.enter_context(tc.tile_pool(name="consts", bufs=1))
        xin = ctx.enter_context(tc.tile_pool(name="xin", bufs=8))
        red = ctx.enter_context(tc.tile_pool(name="red", bufs=1))
        wstr = ctx.enter_context(tc.tile_pool(name="wstr", bufs=12))
        bias_pool = ctx.enter_context(tc.tile_pool(name="bias", bufs=2))
        act_pool = ctx.enter_context(tc.tile_pool(name="act", bufs=1))
        lhsT_pool = ctx.enter_context(tc.tile_pool(name="lhsT", bufs=2))
        psum = ctx.enter_context(tc.tile_pool(name="psum", bufs=4, space="PSUM"))
        psT = ctx.enter_context(tc.tile_pool(name="psT", bufs=3, space="PSUM"))

        ident = consts.tile([128, 128], F32)
        make_identity(nc, ident[:, :])
        ones_t = consts.tile([1, 32], BF16)
        nc.vector.memset(ones_t[:, :], 1.0)

        # ---------------- helpers ----------------
        def wblock(src_ap, nn):
            """Stream one [128, nn<=512] weight block (ACT queue)."""
            blk = wstr.tile([128, 512], BF16, tag="wblk", name="wblk")
            nc.scalar.dma_start(blk[:, :nn], src_ap)
            return blk[:, :nn]

        def load_bias(src, rows, width, tag):
            t = bias_pool.tile([1, rows, width], BF16, tag=tag, name=tag)
            nc.scalar.dma_start(t[0:1, :, :], src[:, :])
            return t

        def mm_layer(lhsT, src_w, nch_k, n_total, bias_ap_fn, epilogue,
                     lhsT2=None, src_w2=None):
            """epilogue(ps, n0, nn) over psum = lhsT.T@W [+ lhsT2.T@W2] + bias."""
            for n0 in range(0, n_total, 512):
                nn = min(512, n_total - n0)
                ps = psum.tile([32, 512], F32, tag="mm", name="mm")
                for kc in range(nch_k):
                    blk = wblock(src_w[kc * 128:(kc + 1) * 128, n0:n0 + nn], nn)
                    nc.tensor.matmul(ps[:, :nn], lhsT[:, kc, :], blk,
                                     start=(kc == 0), stop=False)
                if lhsT2 is not None:
                    for kc in range(nch_k):
                        blk = wblock(src_w2[kc * 128:(kc + 1) * 128, n0:n0 + nn],
                                     nn)
                        nc.tensor.matmul(ps[:, :nn], lhsT2[:, kc, :], blk,
                                         start=False, stop=False)
                nc.tensor.matmul(ps[:, :nn], ones_t[0:1, :], bias_ap_fn(n0, nn),
                                 start=False, stop=True)
                epilogue(ps[:, :nn], n0, nn)

        def act_ep(out_t, func):
            def ep(ps, n0, nn):
                nc.scalar.activation(out_t[:, n0:n0 + nn], ps, func)
            return ep

        def elu_ep(out_t):
            def ep(ps, n0, nn):
                r = act_pool.tile([32, 512], F32, tag="elu_r", name="elu_r",
                                  bufs=2)
                mn = act_pool.tile([32, 512], F32, tag="elu_m", name="elu_m",
                                   bufs=2)
                ex = act_pool.tile([32, 512], F32, tag="elu_e", name="elu_e",
                                   bufs=2)
                nc.scalar.activation(r[:, :nn], ps, AF.Relu)
                nc.vector.tensor_scalar_min(mn[:, :nn], ps, 0.0)
                nc.scalar.activation(ex[:, :nn], mn[:, :nn], AF.Exp)
                nc.vector.scalar_tensor_tensor(out_t[:, n0:n0 + nn], ex[:, :nn],
                                               -1.0, r[:, :nn], op0=ALU.add,
                                               op1=ALU.add)
            return ep

        # ---------------- phase 1: token reductions ----------------
        top1 = {m: red.tile([128, KC, BL], BF16, tag=f"top1_{m}",
                            name=f"top1_{m}") for m in ("p", "g")}
        avgf = {m: red.tile([128, KC, BL], F32, tag=f"avgf_{m}",
                            name=f"avgf_{m}") for m in ("p", "g")}
        avgb = {m: red.tile([128, KC, BL], BF16, tag=f"avgb_{m}",
                            name=f"avgb_{m}") for m in ("p", "g")}

        scr = ctx.enter_context(tc.tile_pool(name="scr", bufs=2))
        for m, x_ext, ntok in (("p", xp, NP_), ("g", xo, NG)):
            for b in range(BL):
                for c in range(KC):
                    t = xin.tile([128, NP_], BF16, tag="xin", name="xin")
                    nc.sync.dma_start_transpose(
                        t[:, :ntok], x_ext[b, :, c * 128:(c + 1) * 128])
                    nc.vector.reduce_max(
                        top1[m][:, c, b:b + 1], t[:, :ntok],
                        axis=mybir.AxisListType.X)
                    sc = scr.tile([128, NP_ - 1], BF16, tag="scr", name="scr")
                    nc.scalar.activation(
                        sc[:, 0:ntok - 1], t[:, 1:ntok], AF.Identity,
                        scale=1.0 / (ntok - 1),
                        accum_out=avgf[m][:, c, b:b + 1])
            nc.vector.tensor_copy(avgb[m][:, :, :], avgf[m][:, :, :])

        # ---------------- ATSA ----------------
        h = {}
        for m in ("p", "g"):
            A = atsa[m]
            b1t = load_bias(A["b1"], 1, D, "ab1")
            m1 = act_pool.tile([32, D], F32, tag="m1", name="m1")
            mm_layer(top1[m], A["w1"], KC, D,
                     lambda n0, nn, t=b1t: t[0:1, 0, n0:n0 + nn],
                     act_ep(m1, AF.Relu))
            # softmax over D (free axis)
            nmax = red.tile([32, 1], F32, tag="nmax", name="nmax")
            nc.vector.reduce_max(nmax[:, :], m1[:, :], axis=mybir.AxisListType.X,
                                 negate=True)
            sexp = act_pool.tile([32, D], F32, tag="sexp", name="sexp")
            ssum = red.tile([32, 1], F32, tag="ssum", name="ssum")
            nc.scalar.activation(sexp[:, :], m1[:, :], AF.Exp, bias=nmax[:, :],
                                 accum_out=ssum[:, :])
            rinv = red.tile([32, 1], F32, tag="rinv", name="rinv")
            nc.vector.reciprocal(rinv[:, :], ssum[:, :])
            nc.vector.tensor_scalar_mul(sexp[:, :], sexp[:, :], rinv[:, :])
            sT = _transpose_to_lhsT(nc, psT, lhsT_pool, ident, sexp, D, "sT")

            bct = load_bias(A["bc"], 1, D, "abc")
            cpre = act_pool.tile([32, D], F32, tag="cpre", name="cpre")
            mm_layer(sT, A["wc"][0:D, :], KC, D,
                     lambda n0, nn, t=bct: t[0:1, 0, n0:n0 + nn],
                     act_ep(cpre, AF.Copy),
                     lhsT2=avgb[m], src_w2=A["wc"][D:2 * D, :])
            cT = _transpose_to_lhsT(nc, psT, lhsT_pool, ident, cpre, D, "cT")

            b2t = load_bias(A["b2"], 1, D, "ab2")
            hm = act_pool.tile([32, D], F32, tag=f"h_{m}", name=f"h_{m}")
            mm_layer(cT, A["w2"], KC, D,
                     lambda n0, nn, t=b2t: t[0:1, 0, n0:n0 + nn],
                     act_ep(hm, AF.Relu))
            h[m] = hm

        gate = {}
        for m, other in (("p", "g"), ("g", "p")):
            gt = act_pool.tile([32, D], F32, tag=f"gate_{m}", name=f"gate_{m}")
            nc.scalar.activation(gt[:, :], h[other][:, :], AF.Sigmoid)
            gate[m] = gt

        # ---------------- PREEG ----------------
        o_out = {}
        for m in ("p", "g"):
            P = preeg[m]
            xT = _transpose_to_lhsT(nc, psT, lhsT_pool, ident, h[m], D, "xT")
            bf1t = load_bias(P["bf1"], 1, DH, "bf1")
            h1 = act_pool.tile([32, DH], F32, tag="h1", name="h1")
            mm_layer(xT, P["wf1"], KC, DH,
                     lambda n0, nn, t=bf1t: t[0:1, 0, n0:n0 + nn],
                     act_ep(h1, AF.Relu))
            h1T = _transpose_to_lhsT(nc, psT, lhsT_pool, ident, h1, DH, "h1T")
            bf2t = load_bias(P["bf2"], 1, D, "bf2")
            outm = act_pool.tile([32, D], F32, tag=f"out_{m}", name=f"out_{m}")
            mm_layer(h1T, P["wf2"], DH // 128, D,
                     lambda n0, nn, t=bf2t: t[0:1, 0, n0:n0 + nn],
                     act_ep(outm, AF.Copy))

            eb1t = load_bias(P["eb1"], E, D, "eb1")
            eb2t = load_bias(P["eb2"], E, D, "eb2")
            for e in range(E):
                oT = _transpose_to_lhsT(nc, psT, lhsT_pool, ident, outm, D, "oT")
                he = act_pool.tile([32, D], F32, tag="he", name="he")
                mm_layer(oT, P["ew1"][e], KC, D,
                         lambda n0, nn, t=eb1t, e=e: t[0:1, e, n0:n0 + nn],
                         elu_ep(he))
                hT = _transpose_to_lhsT(nc, psT, lhsT_pool, ident, he, D, "hT")
                h2 = act_pool.tile([32, D], F32, tag="h2", name="h2")
                mm_layer(hT, P["ew2"][e], KC, D,
                         lambda n0, nn, t=eb2t, e=e: t[0:1, e, n0:n0 + nn],
                         elu_ep(h2))
                gh = act_pool.tile([32, D], F32, tag="gh", name="gh")
                nc.vector.tensor_mul(gh[:, :], gate[m][:, :], h2[:, :])
                nc.vector.tensor_add(outm[:, :], outm[:, :], gh[:, :])
            o_out[m] = outm

        # ---------------- LMF + classifier ----------------
        aT = _transpose_to_lhsT(nc, psT, lhsT_pool, ident, o_out["g"], D, "aT")
        vT = _transpose_to_lhsT(nc, psT, lhsT_pool, ident, o_out["p"], D, "vT")
        fwt = consts.tile([32, R], F32)
        nc.sync.dma_start(fwt[:, :], fwb[:, :])
        befft = consts.tile([32, C], F32)
        nc.sync.dma_start(befft[:, :], beff[:, :])
        fa0 = consts.tile([1, R, O], BF16)
        nc.scalar.dma_start(fa0[0:1, :, :], fa[:, 0, :])
        fv0 = consts.tile([1, R, O], BF16)
        nc.scalar.dma_start(fv0[0:1, :, :], fv[:, 0, :])

        acc = act_pool.tile([32, O], F32, tag="acc", name="acc")
        for r in range(R):
            za = psum.tile([32, 512], F32, tag="mm", name="za")
            zv = psum.tile([32, 512], F32, tag="mm", name="zv")
            for kc in range(KC):
                blk = wblock(fa[r, 1 + kc * 128: 1 + (kc + 1) * 128, :], O)
                nc.tensor.matmul(za[:, :O], aT[:, kc, :], blk,
                                 start=(kc == 0), stop=False)
            nc.tensor.matmul(za[:, :O], ones_t[0:1, :], fa0[0:1, r, :],
                             start=False, stop=True)
            for kc in range(KC):
                blk = wblock(fv[r, 1 + kc * 128: 1 + (kc + 1) * 128, :], O)
                nc.tensor.matmul(zv[:, :O], vT[:, kc, :], blk,
                                 start=(kc == 0), stop=False)
            nc.tensor.matmul(zv[:, :O], ones_t[0:1, :], fv0[0:1, r, :],
                             start=False, stop=True)
            zasb = act_pool.tile([32, O], F32, tag="zasb", name="zasb", bufs=2)
            nc.scalar.activation(zasb[:, :], za[:, :O], AF.Copy)
            t1 = act_pool.tile([32, O], F32, tag="t1", name="t1", bufs=2)
            nc.vector.tensor_mul(t1[:, :], zasb[:, :], zv[:, :O])
            if r == 0:
                nc.vector.tensor_scalar_mul(acc[:, :], t1[:, :], fwt[:, 0:1])
            else:
                nc.vector.scalar_tensor_tensor(acc[:, :], t1[:, :],
                                               fwt[:, r:r + 1], acc[:, :],
                                               op0=ALU.mult, op1=ALU.add)

        fT = _transpose_to_lhsT(nc, psT, lhsT_pool, ident, acc, O, "fT")
        psc = psum.tile([32, 512], F32, tag="mm", name="psc")
        for kc in range(O // 128):
            blk = wblock(cls_w[kc * 128:(kc + 1) * 128, :], C)
            nc.tensor.matmul(psc[:, :C], fT[:, kc, :], blk,
                             start=(kc == 0), stop=(kc == O // 128 - 1))
        res = act_pool.tile([32, C], F32, tag="res", name="res")
        nc.vector.tensor_add(res[:, :], psc[:, :C], befft[:, :])
        nc.sync.dma_start(out_ext[:, :], res[:, :])

    nc.compile()
    return nc


# --------------------------------------------------------------------------
# host wrapper
# --------------------------------------------------------------------------

def _bf16(a):
    return np.ascontiguousarray(np.asarray(a, dtype=np.float32)).astype(
        ml_dtypes.bfloat16)


def kernel(x_path, x_omic, atsa_p, atsa_g, preeg_p, preeg_g, lmf, cls,
           _trace=False):
    if "nc" not in _BUILT:
        _BUILT["nc"] = _build_kernel()
    nc = _BUILT["nc"]

    shared = {}
    for m, A in (("p", atsa_p), ("g", atsa_g)):
        shared[f"a{m}_w1"] = _bf16(A["w1"])
        shared[f"a{m}_b1"] = _bf16(A["b1"]).reshape(1, D)
        shared[f"a{m}_wc"] = _bf16(A["wc"])
        shared[f"a{m}_bc"] = _bf16(A["bc"]).reshape(1, D)
        shared[f"a{m}_w2"] = _bf16(A["w2"])
        shared[f"a{m}_b2"] = _bf16(A["b2"]).reshape(1, D)
    for m, P in (("p", preeg_p), ("g", preeg_g)):
        shared[f"p{m}_wf1"] = _bf16(P["wf1"])
        shared[f"p{m}_bf1"] = _bf16(P["bf1"]).reshape(1, DH)
        shared[f"p{m}_wf2"] = _bf16(P["wf2"])
        shared[f"p{m}_bf2"] = _bf16(P["bf2"]).reshape(1, D)
        shared[f"p{m}_ew1"] = _bf16(P["ew1"])
        shared[f"p{m}_eb1"] = _bf16(P["eb1"])
        shared[f"p{m}_ew2"] = _bf16(P["ew2"])
        shared[f"p{m}_eb2"] = _bf16(P["eb2"])
    shared["lmf_fa"] = _bf16(lmf["fa"])
    shared["lmf_fv"] = _bf16(lmf["fv"])
    fw = np.asarray(lmf["fw"], np.float32).reshape(1, R)
    shared["lmf_fwb"] = np.repeat(fw, BL, axis=0)
    fb = np.asarray(lmf["fb"], np.float32).reshape(1, O)
    cw = np.asarray(cls["w"], np.float32)
    cb = np.asarray(cls["b"], np.float32).reshape(1, C)
    shared["lmf_beff"] = np.repeat(fb @ cw + cb, BL, axis=0).astype(np.float32)
    shared["cls_w"] = _bf16(cw)

    xp_b = _bf16(x_path).reshape(NCORES, BL, NP_, D)
    xo_b = _bf16(x_omic).reshape(NCORES, BL, NG, D)

    in_maps = []
    for s in range(NCORES):
        mp = dict(shared)
        mp["xp"] = xp_b[s]
        mp["xo"] = xo_b[s]
        in_maps.append(mp)

    res = run_bass_kernel_spmd(nc, in_maps, core_ids=list(range(NCORES)),
                               trace=_trace)
    out = np.concatenate([r["out"] for r in res.results], axis=0)
    if _trace:
        kernel.last_exec_time_ns = res.exec_time_ns
        kernel.last_results = res
    return out.astype(np.float32)


# revision 13
# speedup vs baseline: 1.0601x; 1.0601x over previous
"""AdaMHF tower-split kernel for 8 TRN2 NeuronCores.

Phase 1 (token max/mean) is data-parallel over batch (32 samples/core, both
modalities). An AllGather then redistributes the tiny reductions so that
cores 0-3 each run the full pathology tower for 64 samples and cores 4-7 the
genomic tower (weights per core are halved vs pure DP). Gates and the LMF
fusion need the partner tower's activations: two more small AllGathers.
g-cores receive fa/fv swapped by the host, which makes one SPMD graph
compute the same (symmetric) fused product on both towers; the host reads
the final logits from cores 0-3.
"""

import sys
import numpy as np

sys.path.insert(0, "/opt/trn_rl_repo")

import ml_dtypes  # noqa: E402

import concourse.bass as bass  # noqa: E402
import concourse.mybir as mybir  # noqa: E402
import concourse.tile as tile  # noqa: E402
from concourse import bacc  # noqa: E402
from concourse.bass_utils import run_bass_kernel_spmd  # noqa: E402
from concourse.masks import make_identity  # noqa: E402

AF = mybir.ActivationFunctionType
ALU = mybir.AluOpType
BF16 = mybir.dt.bfloat16
F32 = mybir.dt.float32

NCORES = 8
B, BL, BT = 256, 32, 64
D, DH, E, R, O, C = 1024, 4096, 4, 16, 256, 4
NP_, NG = 512, 256
KC = D // 128

_BUILT = {}


def _build_kernel():
    nc = bacc.Bacc("TRN2", target_bir_lowering=False, debug=False,
                   enable_asserts=False, num_devices=NCORES)

    xp = nc.dram_tensor("xp", [BL, NP_, D], BF16, kind="ExternalInput")
    xo = nc.dram_tensor("xo", [BL, NG, D], BF16, kind="ExternalInput")

    def w(name, shape, dt=BF16):
        return nc.dram_tensor(name, shape, dt, kind="ExternalInput")

    A = {"w1": w("a_w1", [D, D]), "b1": w("a_b1", [1, D]),
         "wc": w("a_wc", [2 * D, D]), "bc": w("a_bc", [1, D]),
         "w2": w("a_w2", [D, D]), "b2": w("a_b2", [1, D])}
    P = {"wf1": w("p_wf1", [D, DH]), "bf1": w("p_bf1", [1, DH]),
         "wf2": w("p_wf2", [DH, D]), "bf2": w("p_bf2", [1, D]),
         "ew1": w("p_ew1", [E, D, D]), "eb1": w("p_eb1", [E, D]),
         "ew2": w("p_ew2", [E, D, D]), "eb2": w("p_eb2", [E, D])}
    fa = w("lmf_fa", [R, D + 1, O])
    fv = w("lmf_fv", [R, D + 1, O])
    fwb = w("lmf_fwb", [BT, R], F32)
    beff = w("lmf_beff", [BT, C], F32)
    cls_w = w("cls_w", [O, C])

    out_ext = nc.dram_tensor("out", [BT, C], F32, kind="ExternalOutput")

    import os
    from contextlib import ExitStack
    _ts = os.environ.get("TILE_SIM") == "1"
    with tile.TileContext(nc, trace_sim=_ts) as tc, ExitStack() as ctx:
        consts = ctx.enter_context(tc.tile_pool(name="consts", bufs=1))
        xin = ctx.enter_context(tc.tile_pool(name="xin", bufs=3))
        red = ctx.enter_context(tc.tile_pool(name="red", bufs=1))
        wstr = ctx.enter_context(tc.tile_pool(name="wstr", bufs=4))
        bias_pool = ctx.enter_context(tc.tile_pool(name="bias", bufs=1))
        act_pool = ctx.enter_context(tc.tile_pool(name="act", bufs=1))
        lhsT_pool = ctx.enter_context(tc.tile_pool(name="lhsT", bufs=1))
        dram = ctx.enter_context(tc.tile_pool(name="dram", bufs=1, space="DRAM"))
        psum = ctx.enter_context(tc.tile_pool(name="psum", bufs=4, space="PSUM"))
        psT = ctx.enter_context(tc.tile_pool(name="psT", bufs=3, space="PSUM"))

        ident = consts.tile([128, 128], F32)
        make_identity(nc, ident[:, :])
        identb = consts.tile([128, 128], BF16)
        make_identity(nc, identb[:, :])
        ones_t = consts.tile([1, BT], BF16)
        nc.vector.memset(ones_t[:, :], 1.0)

        def trans_lhsT(src, width, tag, dt=F32):
            """src: [BT, width] sbuf -> bf16 lhsT [128, width//128, BT]."""
            idm = ident if dt == F32 else identb
            nch = width // 128
            dst = lhsT_pool.tile([128, nch, BT], BF16, tag=tag, name=tag)
            for c in range(nch):
                pt = psT.tile([128, BT], dt, tag="pT")
                nc.tensor.transpose(pt[:, :], src[:, c * 128:(c + 1) * 128],
                                    idm[0:BT, 0:BT])
                nc.vector.tensor_copy(dst[:, c, :], pt[:, :])
            return dst

        _wl_count = [0]

        def wtile_load(src2d, koff, nk, n0, nn):
            blk = wstr.tile([128, 8, 512], BF16, tag="wblk", name="wblk")
            ap = src2d[koff:koff + nk * 128, n0:n0 + nn]
            _wl_count[0] += 1
            nc.scalar.dma_start(blk[:, :nk, :nn],
                                ap.rearrange("(c p) n -> p c n", p=128))
            return blk

        def load_bias(src, rows, width, tag):
            t = bias_pool.tile([1, rows, width], BF16, tag=tag, name=tag)
            nc.scalar.dma_start(t[0:1, :, :], src[:, :])
            return t

        def mm_layer(lhsT, src_w, nch_k, n_total, bias_ap_fn, epilogue,
                     lhsT2=None, src_w2=None):
            srcs = [(lhsT, src_w)] + ([(lhsT2, src_w2)] if lhsT2 is not None
                                      else [])
            for n0 in range(0, n_total, 512):
                nn = min(512, n_total - n0)
                ps = psum.tile([BT, 512], F32, tag="mm", name="mm")
                first = True
                for lt, sw in srcs:
                    for kg in range(0, nch_k, 8):
                        nk = min(8, nch_k - kg)
                        blk = wtile_load(sw, kg * 128, nk, n0, nn)
                        for kc in range(nk):
                            nc.tensor.matmul(ps[:, :nn], lt[:, kg + kc, :],
                                             blk[:, kc, :nn],
                                             start=first, stop=False)
                            first = False
                nc.tensor.matmul(ps[:, :nn], ones_t[0:1, :], bias_ap_fn(n0, nn),
                                 start=False, stop=True)
                epilogue(ps[:, :nn], n0, nn)

        def act_ep(out_t, func):
            def ep(ps, n0, nn):
                if func == AF.Relu:
                    nc.vector.tensor_scalar_max(out_t[:, n0:n0 + nn], ps, 0.0)
                elif func == AF.Copy:
                    nc.vector.tensor_copy(out_t[:, n0:n0 + nn], ps)
                else:
                    nc.scalar.activation(out_t[:, n0:n0 + nn], ps, func)
            return ep

        def elu_ep(out_t):
            def ep(ps, n0, nn):
                r = act_pool.tile([BT, 512], F32, tag="elu_r", name="elu_r",
                                  bufs=2)
                mn = act_pool.tile([BT, 512], F32, tag="elu_m", name="elu_m",
                                   bufs=2)
                ex = act_pool.tile([BT, 512], F32, tag="elu_e", name="elu_e",
                                   bufs=2)
                nc.vector.tensor_scalar_max(r[:, :nn], ps, 0.0)
                nc.vector.tensor_scalar_min(mn[:, :nn], ps, 0.0)
                nc.scalar.activation(ex[:, :nn], mn[:, :nn], AF.Exp)
                nc.vector.scalar_tensor_tensor(out_t[:, n0:n0 + nn],
                                               ex[:, :nn], -1.0, r[:, :nn],
                                               op0=ALU.add, op1=ALU.add)
            return ep

        # ---------------- phase 1: token reductions (DP-8) ----------------
        top1 = {m: red.tile([128, KC, BL], BF16, tag=f"top1_{m}",
                            name=f"top1_{m}") for m in ("p", "g")}
        avgf = {m: red.tile([128, KC, BL], F32, tag=f"avgf_{m}",
                            name=f"avgf_{m}") for m in ("p", "g")}
        avgb = {m: red.tile([128, KC, BL], BF16, tag=f"avgb_{m}",
                            name=f"avgb_{m}") for m in ("p", "g")}

        scr = ctx.enter_context(tc.tile_pool(name="scr", bufs=2))
        G = 8
        for m, x_ext, ntok in (("p", xp, NP_), ("g", xo, NG)):
            xflat = x_ext[:, :, :].rearrange("b t d -> (b t) d")
            for c in range(KC):
                for bg in range(0, BL, G):
                    t = xin.tile([128, G * NP_], BF16, tag="xin", name="xin")
                    nc.sync.dma_start_transpose(
                        t[:, 0:G * ntok],
                        xflat[bg * ntok:(bg + G) * ntok,
                              c * 128:(c + 1) * 128])
                    nc.vector.reduce_max(
                        top1[m][:, c, bg:bg + G],
                        t[:, 0:G * ntok].rearrange("p (g t) -> p g t", g=G),
                        axis=mybir.AxisListType.X)
                    if m == "g":
                        # mean over tokens [1:] as grouped DVE reduce; the
                        # 1/(ntok-1) scale is applied in the cast below
                        tv = t[:, 0:G * ntok].rearrange("p (g t) -> p g t",
                                                        g=G)
                        nc.vector.reduce_sum(
                            avgf[m][:, c, bg:bg + G], tv[:, :, 1:ntok],
                            axis=mybir.AxisListType.X)
                    else:
                        for g in range(G):
                            sc = scr.tile([128, NP_ - 1], BF16, tag="scr",
                                          name="scr")
                            nc.scalar.activation(
                                sc[:, 0:ntok - 1],
                                t[:, g * ntok + 1:(g + 1) * ntok], AF.Identity,
                                scale=1.0 / (ntok - 1),
                                accum_out=avgf[m][:, c, bg + g:bg + g + 1])
            if m == "g":
                nc.vector.tensor_scalar_mul(avgb[m][:, :, :],
                                            avgf[m][:, :, :],
                                            1.0 / (NG - 1))
            else:
                nc.vector.tensor_copy(avgb[m][:, :, :], avgf[m][:, :, :])

        # ---------------- AG1: redistribute reductions ----------------
        # layout index: 0=top1_p 1=avg_p 2=top1_g 3=avg_g
        redin = dram.tile([4, 128, KC, BL], BF16)
        for i, tl in enumerate((top1["p"], avgb["p"], top1["g"], avgb["g"])):
            nc.sync.dma_start(redin[i, :, :, :], tl[:, :, :])
        redout = dram.tile([NCORES, 4, 128, KC, BL], BF16, addr_space="Shared")
        nc.gpsimd.collective_compute(
            "AllGather", ALU.bypass, replica_groups=[list(range(NCORES))],
            ins=[redin.opt()], outs=[redout.opt()])

        pid = nc.partition_id()
        j = pid % 4          # pair index -> samples 64j..64j+64
        tw = pid // 4        # tower: 0=pathology 1=genomic
        top1_my = red.tile([128, KC, BT], BF16, tag="top1_my", name="top1_my")
        avg_my = red.tile([128, KC, BT], BF16, tag="avg_my", name="avg_my")
        for h in range(2):
            src = 2 * j + h
            nc.sync.dma_start(
                top1_my[:, :, h * BL:(h + 1) * BL],
                redout[bass.ds(src, 1), bass.ds(2 * tw, 1), :, :, :])
            nc.sync.dma_start(
                avg_my[:, :, h * BL:(h + 1) * BL],
                redout[bass.ds(src, 1), bass.ds(2 * tw + 1, 1), :, :, :])

        # ---------------- ATSA (own tower, 64 samples) ----------------
        b1t = load_bias(A["b1"], 1, D, "ab1")
        m1 = act_pool.tile([BT, D], F32, tag="m1", name="m1")
        mm_layer(top1_my, A["w1"], KC, D,
                 lambda n0, nn, t=b1t: t[0:1, 0, n0:n0 + nn],
                 act_ep(m1, AF.Relu))
        nmax = red.tile([BT, 1], F32, tag="nmax", name="nmax")
        nc.vector.reduce_max(nmax[:, :], m1[:, :], axis=mybir.AxisListType.X,
                             negate=True)
        sexp = act_pool.tile([BT, D], F32, tag="sexp", name="sexp")
        ssum = red.tile([BT, 1], F32, tag="ssum", name="ssum")
        nc.scalar.activation(sexp[:, :], m1[:, :], AF.Exp, bias=nmax[:, :],
                             accum_out=ssum[:, :])
        rinv = red.tile([BT, 1], F32, tag="rinv", name="rinv")
        nc.vector.reciprocal(rinv[:, :], ssum[:, :])
        nc.vector.tensor_scalar_mul(sexp[:, :], sexp[:, :], rinv[:, :])
        sT = trans_lhsT(sexp, D, "sT")

        bct = load_bias(A["bc"], 1, D, "abc")
        cpre = act_pool.tile([BT, D], F32, tag="cpre", name="cpre")
        mm_layer(sT, A["wc"][0:D, :], KC, D,
                 lambda n0, nn, t=bct: t[0:1, 0, n0:n0 + nn],
                 act_ep(cpre, AF.Copy),
                 lhsT2=avg_my, src_w2=A["wc"][D:2 * D, :])
        cT = trans_lhsT(cpre, D, "cT")

        b2t = load_bias(A["b2"], 1, D, "ab2")
        hm = act_pool.tile([BT, D], F32, tag="hm", name="hm")
        mm_layer(cT, A["w2"], KC, D,
                 lambda n0, nn, t=b2t: t[0:1, 0, n0:n0 + nn],
                 act_ep(hm, AF.Relu))

        # ---------------- AG2: exchange h for gates ----------------
        hm_bf = act_pool.tile([BT, D], BF16, tag="hm_bf", name="hm_bf")
        nc.vector.tensor_copy(hm_bf[:, :], hm[:, :])
        hin = dram.tile([BT, D], BF16)
        nc.sync.dma_start(hin[:, :], hm_bf[:, :])
        hout = dram.tile([NCORES, BT, D], BF16, addr_space="Shared")
        nc.gpsimd.collective_compute(
            "AllGather", ALU.bypass, replica_groups=[list(range(NCORES))],
            ins=[hin.opt()], outs=[hout.opt()])
        partner = (pid + 4) % 8
        hpart = act_pool.tile([BT, D], BF16, tag="hpart", name="hpart")
        nc.sync.dma_start(hpart[:, :], hout[bass.ds(partner, 1), :, :])
        gate = act_pool.tile([BT, D], F32, tag="gate", name="gate")
        nc.scalar.activation(gate[:, :], hpart[:, :], AF.Sigmoid)

        # ---------------- PREEG (own tower) ----------------
        xT = trans_lhsT(hm, D, "xT")
        bf1t = load_bias(P["bf1"], 1, DH, "bf1")
        h1 = act_pool.tile([BT, DH], F32, tag="h1", name="h1")
        mm_layer(xT, P["wf1"], KC, DH,
                 lambda n0, nn, t=bf1t: t[0:1, 0, n0:n0 + nn],
                 act_ep(h1, AF.Relu))
        h1T = trans_lhsT(h1, DH, "h1T")
        bf2t = load_bias(P["bf2"], 1, D, "bf2")
        outm = act_pool.tile([BT, D], F32, tag="outm", name="outm")
        mm_layer(h1T, P["wf2"], DH // 128, D,
                 lambda n0, nn, t=bf2t: t[0:1, 0, n0:n0 + nn],
                 act_ep(outm, AF.Copy))

        eb1t = load_bias(P["eb1"], E, D, "eb1")
        eb2t = load_bias(P["eb2"], E, D, "eb2")
        for e in range(E):
            oT = trans_lhsT(outm, D, "oT")
            he = act_pool.tile([BT, D], F32, tag="he", name="he")
            mm_layer(oT, P["ew1"][e], KC, D,
                     lambda n0, nn, t=eb1t, e=e: t[0:1, e, n0:n0 + nn],
                     elu_ep(he))
            hT = trans_lhsT(he, D, "hT")
            h2 = act_pool.tile([BT, D], F32, tag="h2", name="h2")
            mm_layer(hT, P["ew2"][e], KC, D,
                     lambda n0, nn, t=eb2t, e=e: t[0:1, e, n0:n0 + nn],
                     elu_ep(h2))
            gh = act_pool.tile([BT, D], F32, tag="gh", name="gh")
            nc.vector.tensor_mul(gh[:, :], gate[:, :], h2[:, :])
            nc.vector.tensor_add(outm[:, :], outm[:, :], gh[:, :])

        # ---------------- AG3: exchange tower outputs ----------------
        o_bf = act_pool.tile([BT, D], BF16, tag="o_bf", name="o_bf")
        nc.vector.tensor_copy(o_bf[:, :], outm[:, :])
        oin = dram.tile([BT, D], BF16)
        nc.sync.dma_start(oin[:, :], o_bf[:, :])
        oout = dram.tile([NCORES, BT, D], BF16, addr_space="Shared")
        nc.gpsimd.collective_compute(
            "AllGather", ALU.bypass, replica_groups=[list(range(NCORES))],
            ins=[oin.opt()], outs=[oout.opt()])
        other_bf = act_pool.tile([BT, D], BF16, tag="other_bf", name="other_bf")
        nc.sync.dma_start(other_bf[:, :], oout[bass.ds(partner, 1), :, :])

        # ---------------- LMF + classifier ----------------
        # z1 = "fa" x other, z2 = "fv" x own; zy = z1*z2 is tower-symmetric
        otherT = trans_lhsT(other_bf, D, "otherT", dt=BF16)
        ownT = trans_lhsT(o_bf, D, "ownT", dt=BF16)
        fwt = consts.tile([BT, R], F32)
        nc.sync.dma_start(fwt[:, :], fwb[:, :])
        befft = consts.tile([BT, C], F32)
        nc.sync.dma_start(befft[:, :], beff[:, :])
        fa0 = consts.tile([1, R, O], BF16)
        nc.scalar.dma_start(fa0[0:1, :, :], fa[:, 0, :])
        fv0 = consts.tile([1, R, O], BF16)
        nc.scalar.dma_start(fv0[0:1, :, :], fv[:, 0, :])

        acc = act_pool.tile([BT, O], F32, tag="acc", name="acc")
        for r in range(R):
            za = psum.tile([BT, 512], F32, tag="mm", name="za")
            zv = psum.tile([BT, 512], F32, tag="mm", name="zv")
            fat = wtile_load(fa[r], 1, KC, 0, O)
            fvt = wtile_load(fv[r], 1, KC, 0, O)
            for kc in range(KC):
                nc.tensor.matmul(za[:, :O], otherT[:, kc, :], fat[:, kc, :O],
                                 start=(kc == 0), stop=False)
            nc.tensor.matmul(za[:, :O], ones_t[0:1, :], fa0[0:1, r, :],
                             start=False, stop=True)
            for kc in range(KC):
                nc.tensor.matmul(zv[:, :O], ownT[:, kc, :], fvt[:, kc, :O],
                                 start=(kc == 0), stop=False)
            nc.tensor.matmul(zv[:, :O], ones_t[0:1, :], fv0[0:1, r, :],
                             start=False, stop=True)
            zasb = act_pool.tile([BT, O], F32, tag="zasb", name="zasb", bufs=2)
            nc.scalar.activation(zasb[:, :], za[:, :O], AF.Copy)
            t1 = act_pool.tile([BT, O], F32, tag="t1", name="t1", bufs=2)
            nc.vector.tensor_mul(t1[:, :], zasb[:, :], zv[:, :O])
            if r == 0:
                nc.vector.tensor_scalar_mul(acc[:, :], t1[:, :], fwt[:, 0:1])
            else:
                nc.vector.scalar_tensor_tensor(acc[:, :], t1[:, :],
                                               fwt[:, r:r + 1], acc[:, :],
                                               op0=ALU.mult, op1=ALU.add)

        fT = trans_lhsT(acc, O, "fT")
        psc = psum.tile([BT, 512], F32, tag="mm", name="psc")
        cwt = wtile_load(cls_w, 0, O // 128, 0, C)
        for kc in range(O // 128):
            nc.tensor.matmul(psc[:, :C], fT[:, kc, :], cwt[:, kc, :C],
                             start=(kc == 0), stop=(kc == O // 128 - 1))
        res = act_pool.tile([BT, C], F32, tag="res", name="res")
        nc.vector.tensor_add(res[:, :], psc[:, :C], befft[:, :])
        nc.sync.dma_start(out_ext[:, :], res[:, :])

    nc.compile()
    return nc


def _bf16(a):
    return np.ascontiguousarray(np.asarray(a, dtype=np.float32)).astype(
        ml_dtypes.bfloat16)


def kernel(x_path, x_omic, atsa_p, atsa_g, preeg_p, preeg_g, lmf, cls,
           _trace=False):
    if "nc" not in _BUILT:
        _BUILT["nc"] = _build_kernel()
    nc = _BUILT["nc"]

    def atsa_map(A):
        return {"a_w1": _bf16(A["w1"]), "a_b1": _bf16(A["b1"]).reshape(1, D),
                "a_wc": _bf16(A["wc"]), "a_bc": _bf16(A["bc"]).reshape(1, D),
                "a_w2": _bf16(A["w2"]), "a_b2": _bf16(A["b2"]).reshape(1, D)}

    def preeg_map(P):
        return {"p_wf1": _bf16(P["wf1"]),
                "p_bf1": _bf16(P["bf1"]).reshape(1, DH),
                "p_wf2": _bf16(P["wf2"]),
                "p_bf2": _bf16(P["bf2"]).reshape(1, D),
                "p_ew1": _bf16(P["ew1"]), "p_eb1": _bf16(P["eb1"]),
                "p_ew2": _bf16(P["ew2"]), "p_eb2": _bf16(P["eb2"])}

    fa_b, fv_b = _bf16(lmf["fa"]), _bf16(lmf["fv"])
    fw = np.asarray(lmf["fw"], np.float32).reshape(1, R)
    fwb = np.repeat(fw, BT, axis=0)
    fb = np.asarray(lmf["fb"], np.float32).reshape(1, O)
    cw = np.asarray(cls["w"], np.float32)
    cb = np.asarray(cls["b"], np.float32).reshape(1, C)
    beff = np.repeat(fb @ cw + cb, BT, axis=0).astype(np.float32)
    cwb = _bf16(cw)

    tower_p = {**atsa_map(atsa_p), **preeg_map(preeg_p),
               "lmf_fa": fa_b, "lmf_fv": fv_b}
    tower_g = {**atsa_map(atsa_g), **preeg_map(preeg_g),
               "lmf_fa": fv_b, "lmf_fv": fa_b}   # swapped on purpose
    common = {"lmf_fwb": fwb, "lmf_beff": beff, "cls_w": cwb}

    xp_b = _bf16(x_path).reshape(NCORES, BL, NP_, D)
    xo_b = _bf16(x_omic).reshape(NCORES, BL, NG, D)

    in_maps = []
    for s in range(NCORES):
        tower = tower_p if s < 4 else tower_g
        mp = dict(tower)
        mp.update(common)
        mp["xp"] = xp_b[s]
        mp["xo"] = xo_b[s]
        in_maps.append(mp)

    kernel.last_in_maps = in_maps
    res = run_bass_kernel_spmd(nc, in_maps, core_ids=list(range(NCORES)),
                               trace=_trace)
    out = np.concatenate([res.results[s]["out"] for s in range(4)], axis=0)
    if _trace:
        kernel.last_exec_time_ns = res.exec_time_ns
        kernel.last_results = res
    return out.astype(np.float32)


# revision 15
# speedup vs baseline: 1.5402x; 1.4529x over previous
"""AdaMHF tower-split kernel for 8 TRN2 NeuronCores.

Phase 1 (token max/mean) is data-parallel over batch (32 samples/core, both
modalities). An AllGather then redistributes the tiny reductions so that
cores 0-3 each run the full pathology tower for 64 samples and cores 4-7 the
genomic tower (weights per core are halved vs pure DP). Gates and the LMF
fusion need the partner tower's activations: two more small AllGathers.
g-cores receive fa/fv swapped by the host, which makes one SPMD graph
compute the same (symmetric) fused product on both towers; the host reads
the final logits from cores 0-3.
"""

import sys
import numpy as np

sys.path.insert(0, "/opt/trn_rl_repo")

import ml_dtypes  # noqa: E402

import concourse.bass as bass  # noqa: E402
import concourse.mybir as mybir  # noqa: E402
import concourse.tile as tile  # noqa: E402
from concourse import bacc  # noqa: E402
from concourse.bass_utils import run_bass_kernel_spmd  # noqa: E402
from concourse.masks import make_identity  # noqa: E402

AF = mybir.ActivationFunctionType
ALU = mybir.AluOpType
BF16 = mybir.dt.bfloat16
F32 = mybir.dt.float32

NCORES = 8
B, BL, BT = 256, 32, 64
D, DH, E, R, O, C = 1024, 4096, 4, 16, 256, 4
NP_, NG = 512, 256
KC = D // 128

_BUILT = {}


def _build_kernel():
    nc = bacc.Bacc("TRN2", target_bir_lowering=False, debug=False,
                   enable_asserts=False, num_devices=NCORES)

    xp = nc.dram_tensor("xp", [BL, NP_, D], BF16, kind="ExternalInput")
    xo = nc.dram_tensor("xo", [BL, NG, D], BF16, kind="ExternalInput")

    def w(name, shape, dt=BF16):
        return nc.dram_tensor(name, shape, dt, kind="ExternalInput")

    A = {"w1": w("a_w1", [D, D]), "b1": w("a_b1", [1, D]),
         "wc": w("a_wc", [2 * D, D]), "bc": w("a_bc", [1, D]),
         "w2": w("a_w2", [D, D]), "b2": w("a_b2", [1, D])}
    P = {"wf1": w("p_wf1", [D, DH]), "bf1": w("p_bf1", [1, DH]),
         "wf2": w("p_wf2", [DH, D]), "bf2": w("p_bf2", [1, D]),
         "ew1": w("p_ew1", [E, D, D]), "eb1": w("p_eb1", [E, D]),
         "ew2": w("p_ew2", [E, D, D]), "eb2": w("p_eb2", [E, D])}
    fa = w("lmf_fa", [R, D + 1, O])
    fv = w("lmf_fv", [R, D + 1, O])
    fwb = w("lmf_fwb", [BT, R], F32)
    beff = w("lmf_beff", [BT, C], F32)
    cls_w = w("cls_w", [O, C])

    out_ext = nc.dram_tensor("out", [BT, C], F32, kind="ExternalOutput")

    import os
    from contextlib import ExitStack
    _ts = os.environ.get("TILE_SIM") == "1"
    with tile.TileContext(nc, trace_sim=_ts) as tc, ExitStack() as ctx:
        consts = ctx.enter_context(tc.tile_pool(name="consts", bufs=1))
        xin = ctx.enter_context(tc.tile_pool(name="xin", bufs=4))
        red = ctx.enter_context(tc.tile_pool(name="red", bufs=1))
        wstr = ctx.enter_context(tc.tile_pool(name="wstr", bufs=7))
        bias_pool = ctx.enter_context(tc.tile_pool(name="bias", bufs=1))
        act_pool = ctx.enter_context(tc.tile_pool(name="act", bufs=1))
        lhsT_pool = ctx.enter_context(tc.tile_pool(name="lhsT", bufs=1))
        dram = ctx.enter_context(tc.tile_pool(name="dram", bufs=1, space="DRAM"))
        psum = ctx.enter_context(tc.tile_pool(name="psum", bufs=4, space="PSUM"))
        psT = ctx.enter_context(tc.tile_pool(name="psT", bufs=2, space="PSUM"))

        ident = consts.tile([128, 128], F32)
        make_identity(nc, ident[:, :])
        identb = consts.tile([128, 128], BF16)
        make_identity(nc, identb[:, :])
        ones_t = consts.tile([1, BT], BF16)
        nc.vector.memset(ones_t[:, :], 1.0)

        def trans_lhsT(src, width, tag, dt=F32):
            """src: [BT, width] sbuf -> bf16 lhsT [128, width//128, BT]."""
            idm = ident if dt == F32 else identb
            nch = width // 128
            dst = lhsT_pool.tile([128, nch, BT], BF16, tag=tag, name=tag)
            for c in range(nch):
                pt = psT.tile([128, BT], dt, tag="pT")
                nc.tensor.transpose(pt[:, :], src[:, c * 128:(c + 1) * 128],
                                    idm[0:BT, 0:BT])
                nc.vector.tensor_copy(dst[:, c, :], pt[:, :])
            return dst

        _wl_count = [0]

        def wtile_load(src2d, koff, nk, n0, nn):
            blk = wstr.tile([128, 8, 512], BF16, tag="wblk", name="wblk")
            ap = src2d[koff:koff + nk * 128, n0:n0 + nn]
            _wl_count[0] += 1
            nc.scalar.dma_start(blk[:, :nk, :nn],
                                ap.rearrange("(c p) n -> p c n", p=128))
            return blk

        def load_bias(src, rows, width, tag):
            t = bias_pool.tile([1, rows, width], BF16, tag=tag, name=tag)
            nc.scalar.dma_start(t[0:1, :, :], src[:, :])
            return t

        def mm_layer(lhsT, src_w, nch_k, n_total, bias_ap_fn, epilogue,
                     lhsT2=None, src_w2=None):
            srcs = [(lhsT, src_w)] + ([(lhsT2, src_w2)] if lhsT2 is not None
                                      else [])
            for n0 in range(0, n_total, 512):
                nn = min(512, n_total - n0)
                ps = psum.tile([BT, 512], F32, tag="mm", name="mm", bufs=3)
                first = True
                for lt, sw in srcs:
                    for kg in range(0, nch_k, 8):
                        nk = min(8, nch_k - kg)
                        blk = wtile_load(sw, kg * 128, nk, n0, nn)
                        for kc in range(nk):
                            nc.tensor.matmul(ps[:, :nn], lt[:, kg + kc, :],
                                             blk[:, kc, :nn],
                                             start=first, stop=False)
                            first = False
                nc.tensor.matmul(ps[:, :nn], ones_t[0:1, :], bias_ap_fn(n0, nn),
                                 start=False, stop=True)
                epilogue(ps[:, :nn], n0, nn)

        def mm_layer_T(w_src, rhsT, nch_k, dout_total, bias_t, bias_row,
                       epilogue_T):
            """T-layout layer: psum[do(128), col(8)*BT] per 1024-dout bank.

            w_src: [nch_k*128, dout_total] dram; rhsT: [128, nch_k, BT] bf16;
            epilogue_T(ps, dg) consumes one [128, 8*BT] bank covering douts
            [dg, dg+1024)."""
            for dg in range(0, dout_total, 1024):
                ps = psum.tile([128, 8 * BT], F32, tag="mmT", name="mmT",
                               bufs=3)
                for half in range(2):
                    n0 = dg + half * 512
                    blks = [wtile_load(w_src, kg * 128, min(8, nch_k - kg),
                                       n0, 512)
                            for kg in range(0, nch_k, 8)]
                    for dc in range(4):
                        col = half * 4 + dc
                        cs = ps[:, col * BT:(col + 1) * BT]
                        first = True
                        for gi, kg in enumerate(range(0, nch_k, 8)):
                            for kc in range(min(8, nch_k - kg)):
                                nc.tensor.matmul(
                                    cs, blks[gi][:, kc,
                                                 dc * 128:(dc + 1) * 128],
                                    rhsT[:, kg + kc, :],
                                    start=first, stop=False)
                                first = False
                        nc.tensor.matmul(
                            cs, bias_t[0:1, bias_row,
                                       n0 + dc * 128:n0 + (dc + 1) * 128],
                            ones_t[0:1, :], start=False, stop=True)
                epilogue_T(ps, dg)

        def act_ep(out_t, func):
            def ep(ps, n0, nn):
                if func == AF.Relu:
                    nc.vector.tensor_scalar_max(out_t[:, n0:n0 + nn], ps, 0.0)
                elif func == AF.Copy:
                    nc.vector.tensor_copy(out_t[:, n0:n0 + nn], ps)
                else:
                    nc.scalar.activation(out_t[:, n0:n0 + nn], ps, func)
            return ep

        def elu_ep(out_t):
            def ep(ps, n0, nn):
                r = act_pool.tile([BT, 512], F32, tag="elu_r", name="elu_r",
                                  bufs=2)
                mn = act_pool.tile([BT, 512], F32, tag="elu_m", name="elu_m",
                                   bufs=2)
                ex = act_pool.tile([BT, 512], F32, tag="elu_e", name="elu_e",
                                   bufs=2)
                nc.vector.tensor_scalar_max(r[:, :nn], ps, 0.0)
                nc.vector.tensor_scalar_min(mn[:, :nn], ps, 0.0)
                nc.scalar.activation(ex[:, :nn], mn[:, :nn], AF.Exp)
                nc.vector.scalar_tensor_tensor(out_t[:, n0:n0 + nn],
                                               ex[:, :nn], -1.0, r[:, :nn],
                                               op0=ALU.add, op1=ALU.add)
            return ep

        # ---------------- phase 1: token reductions (DP-8) ----------------
        top1 = {m: red.tile([128, KC, BL], BF16, tag=f"top1_{m}",
                            name=f"top1_{m}") for m in ("p", "g")}
        avgf = {m: red.tile([128, KC, BL], F32, tag=f"avgf_{m}",
                            name=f"avgf_{m}") for m in ("p", "g")}
        avgb = {m: red.tile([128, KC, BL], BF16, tag=f"avgb_{m}",
                            name=f"avgb_{m}") for m in ("p", "g")}

        scr = ctx.enter_context(tc.tile_pool(name="scr", bufs=2))
        G = 8

        def phase1_job(m, xflat, ntok, c, bg):
            t = xin.tile([128, G * NP_], BF16, tag="xin", name="xin")
            nc.sync.dma_start_transpose(
                t[:, 0:G * ntok],
                xflat[bg * ntok:(bg + G) * ntok, c * 128:(c + 1) * 128])
            tv = t[:, 0:G * ntok].rearrange("p (g t) -> p g t", g=G)
            msc = scr.tile([128, G, NP_ // 2], BF16, tag="msc", name="msc",
                           bufs=2)
            h = ntok // 2
            nc.vector.tensor_max(msc[:, :, 0:h], tv[:, :, 0:h],
                                 tv[:, :, h:2 * h])
            while h > 2:
                h //= 2
                nc.vector.tensor_max(msc[:, :, 0:h], msc[:, :, 0:h],
                                     msc[:, :, h:2 * h])
            nc.vector.tensor_max(top1[m][:, c, bg:bg + G],
                                 msc[:, :, 0:1], msc[:, :, 1:2])
            if m == "g":
                # mean over tokens [1:] as grouped DVE reduce; the
                # 1/(ntok-1) scale is applied in the cast below
                nc.vector.reduce_sum(
                    avgf[m][:, c, bg:bg + G], tv[:, :, 1:ntok],
                    axis=mybir.AxisListType.X)
            else:
                for g in range(G):
                    sc = scr.tile([128, NP_ - 1], BF16, tag="scr", name="scr")
                    nc.scalar.activation(
                        sc[:, 0:ntok - 1],
                        t[:, g * ntok + 1:(g + 1) * ntok], AF.Identity,
                        scale=1.0,
                        accum_out=avgf[m][:, c, bg + g:bg + g + 1])

        # interleave p and g so ACT (path means) and DVE (omic means)
        # stay concurrently busy instead of phase-sequential
        xpf = xp[:, :, :].rearrange("b t d -> (b t) d")
        xgf = xo[:, :, :].rearrange("b t d -> (b t) d")
        jobs_p = [("p", xpf, NP_, c, bg)
                  for c in range(KC) for bg in range(0, BL, G)]
        jobs_g = [("g", xgf, NG, c, bg)
                  for c in range(KC) for bg in range(0, BL, G)]
        for jp, jg in zip(jobs_p, jobs_g):
            phase1_job(*jp)
            phase1_job(*jg)
        nc.vector.tensor_scalar_mul(avgb["p"][:, :, :], avgf["p"][:, :, :],
                                    1.0 / (NP_ - 1))
        nc.vector.tensor_scalar_mul(avgb["g"][:, :, :], avgf["g"][:, :, :],
                                    1.0 / (NG - 1))

        # ---------------- AG1: redistribute reductions ----------------
        # layout index: 0=top1_p 1=avg_p 2=top1_g 3=avg_g
        redin = dram.tile([4, 128, KC, BL], BF16)
        for i, tl in enumerate((top1["p"], avgb["p"], top1["g"], avgb["g"])):
            nc.sync.dma_start(redin[i, :, :, :], tl[:, :, :])
        redout = dram.tile([NCORES, 4, 128, KC, BL], BF16, addr_space="Shared")
        nc.gpsimd.collective_compute(
            "AllGather", ALU.bypass, replica_groups=[list(range(NCORES))],
            ins=[redin.opt()], outs=[redout.opt()])

        pid = nc.partition_id()
        j = pid % 4          # pair index -> samples 64j..64j+64
        tw = pid // 4        # tower: 0=pathology 1=genomic
        top1_my = red.tile([128, KC, BT], BF16, tag="top1_my", name="top1_my")
        avg_my = red.tile([128, KC, BT], BF16, tag="avg_my", name="avg_my")
        for h in range(2):
            src = 2 * j + h
            nc.sync.dma_start(
                top1_my[:, :, h * BL:(h + 1) * BL],
                redout[bass.ds(src, 1), bass.ds(2 * tw, 1), :, :, :])
            nc.sync.dma_start(
                avg_my[:, :, h * BL:(h + 1) * BL],
                redout[bass.ds(src, 1), bass.ds(2 * tw + 1, 1), :, :, :])

        # ---------------- ATSA (own tower, 64 samples) ----------------
        b1t = load_bias(A["b1"], 1, D, "ab1")
        m1 = act_pool.tile([BT, D], F32, tag="m1", name="m1")
        mm_layer(top1_my, A["w1"], KC, D,
                 lambda n0, nn, t=b1t: t[0:1, 0, n0:n0 + nn],
                 act_ep(m1, AF.Relu))
        nmax = red.tile([BT, 1], F32, tag="nmax", name="nmax")
        nc.vector.reduce_max(nmax[:, :], m1[:, :], axis=mybir.AxisListType.X,
                             negate=True)
        sexp = act_pool.tile([BT, D], F32, tag="sexp", name="sexp")
        ssum = red.tile([BT, 1], F32, tag="ssum", name="ssum")
        nc.scalar.activation(sexp[:, :], m1[:, :], AF.Exp, bias=nmax[:, :],
                             accum_out=ssum[:, :])
        rinv = red.tile([BT, 1], F32, tag="rinv", name="rinv")
        nc.vector.reciprocal(rinv[:, :], ssum[:, :])
        nc.vector.tensor_scalar_mul(sexp[:, :], sexp[:, :], rinv[:, :])
        sT = trans_lhsT(sexp, D, "sT")

        bct = load_bias(A["bc"], 1, D, "abc")
        cpre = act_pool.tile([BT, D], F32, tag="cpre", name="cpre")
        mm_layer(sT, A["wc"][0:D, :], KC, D,
                 lambda n0, nn, t=bct: t[0:1, 0, n0:n0 + nn],
                 act_ep(cpre, AF.Copy),
                 lhsT2=avg_my, src_w2=A["wc"][D:2 * D, :])
        cT = trans_lhsT(cpre, D, "cT")

        b2t = load_bias(A["b2"], 1, D, "ab2")
        hm = act_pool.tile([BT, D], F32, tag="hm", name="hm")
        mm_layer(cT, A["w2"], KC, D,
                 lambda n0, nn, t=b2t: t[0:1, 0, n0:n0 + nn],
                 act_ep(hm, AF.Relu))

        # ---------------- AG2: exchange hT for gates ----------------
        xT = trans_lhsT(hm, D, "xT")     # also the FixedMLP rhs
        hin = dram.tile([128, KC, BT], BF16)
        nc.sync.dma_start(hin[:, :, :], xT[:, :, :])
        hout = dram.tile([NCORES, 128, KC, BT], BF16, addr_space="Shared")
        nc.gpsimd.collective_compute(
            "AllGather", ALU.bypass, replica_groups=[list(range(NCORES))],
            ins=[hin.opt()], outs=[hout.opt()])
        partner = (pid + 4) % 8
        hpartT = act_pool.tile([128, KC, BT], BF16, tag="hpartT",
                               name="hpartT")
        nc.sync.dma_start(hpartT[:, :, :], hout[bass.ds(partner, 1), :, :, :])
        gateT = act_pool.tile([128, KC, BT], F32, tag="gateT", name="gateT")
        nc.scalar.activation(gateT[:, :, :], hpartT[:, :, :], AF.Sigmoid)

        # ---------------- PREEG (own tower, T-layout) ----------------
        def relu_epT(out_T):
            def ep(ps, dg):
                nc.vector.tensor_scalar_max(
                    out_T[:, dg // 128:dg // 128 + 8, :].rearrange(
                        "p c b -> p (c b)"), ps[:, :], 0.0)
            return ep

        def elu_epT(out_T, out_is_3d=True):
            def ep(ps, dg):
                r = act_pool.tile([128, 8 * BT], F32, tag="elu_r",
                                  name="elu_r", bufs=2)
                mn = act_pool.tile([128, 8 * BT], F32, tag="elu_m",
                                   name="elu_m", bufs=2)
                ex = act_pool.tile([128, 8 * BT], F32, tag="elu_e",
                                   name="elu_e", bufs=2)
                nc.vector.tensor_scalar_max(r[:, :], ps[:, :], 0.0)
                nc.vector.tensor_scalar_min(mn[:, :], ps[:, :], 0.0)
                nc.scalar.activation(ex[:, :], mn[:, :], AF.Exp)
                dst = (out_T[:, dg // 128:dg // 128 + 8, :].rearrange(
                    "p c b -> p (c b)") if out_is_3d else out_T[:, :])
                nc.vector.scalar_tensor_tensor(dst, ex[:, :], -1.0, r[:, :],
                                               op0=ALU.add, op1=ALU.add)
            return ep

        bf1t = load_bias(P["bf1"], 1, DH, "bf1")
        h1T = lhsT_pool.tile([128, DH // 128, BT], BF16, tag="h1T",
                             name="h1T")
        mm_layer_T(P["wf1"], xT, KC, DH, bf1t, 0, relu_epT(h1T))
        bf2t = load_bias(P["bf2"], 1, D, "bf2")
        outT = act_pool.tile([128, KC, BT], F32, tag="outT", name="outT")

        def copy_epT(out_T):
            def ep(ps, dg):
                nc.vector.tensor_copy(
                    out_T[:, dg // 128:dg // 128 + 8, :].rearrange(
                        "p c b -> p (c b)"), ps[:, :])
            return ep

        mm_layer_T(P["wf2"], h1T, DH // 128, D, bf2t, 0, copy_epT(outT))

        eb1t = load_bias(P["eb1"], E, D, "eb1")
        eb2t = load_bias(P["eb2"], E, D, "eb2")
        outT_flat = outT[:, :, :].rearrange("p c b -> p (c b)")
        gateT_flat = gateT[:, :, :].rearrange("p c b -> p (c b)")
        for e in range(E):
            o_rhs = lhsT_pool.tile([128, KC, BT], BF16, tag="o_rhs",
                                   name="o_rhs", bufs=2)
            nc.vector.tensor_copy(
                o_rhs[:, :, :].rearrange("p c b -> p (c b)"), outT_flat)
            heT = lhsT_pool.tile([128, KC, BT], BF16, tag="heT", name="heT")
            mm_layer_T(P["ew1"][e], o_rhs, KC, D, eb1t, e, elu_epT(heT))
            h2sb = act_pool.tile([128, 8 * BT], F32, tag="h2sb", name="h2sb")
            mm_layer_T(P["ew2"][e], heT, KC, D, eb2t, e,
                       elu_epT(h2sb, out_is_3d=False))
            gh = act_pool.tile([128, 8 * BT], F32, tag="gh", name="gh")
            nc.vector.tensor_mul(gh[:, :], gateT_flat, h2sb[:, :])
            nc.vector.tensor_add(outT_flat, outT_flat, gh[:, :])

        # ---------------- AG3: exchange tower outputs (T-layout) --------
        o_bfT = lhsT_pool.tile([128, KC, BT], BF16, tag="o_bfT", name="o_bfT")
        nc.vector.tensor_copy(
            o_bfT[:, :, :].rearrange("p c b -> p (c b)"), outT_flat)
        oin = dram.tile([128, KC, BT], BF16)
        nc.sync.dma_start(oin[:, :, :], o_bfT[:, :, :])
        oout = dram.tile([NCORES, 128, KC, BT], BF16, addr_space="Shared")
        nc.gpsimd.collective_compute(
            "AllGather", ALU.bypass, replica_groups=[list(range(NCORES))],
            ins=[oin.opt()], outs=[oout.opt()])
        otherT = lhsT_pool.tile([128, KC, BT], BF16, tag="otherT",
                                name="otherT")
        nc.sync.dma_start(otherT[:, :, :], oout[bass.ds(partner, 1), :, :, :])

        # ---------------- LMF + classifier ----------------
        # z1 = "fa" x other, z2 = "fv" x own; zy = z1*z2 is tower-symmetric
        ownT = o_bfT
        fwt = consts.tile([BT, R], F32)
        nc.sync.dma_start(fwt[:, :], fwb[:, :])
        befft = consts.tile([BT, C], F32)
        nc.sync.dma_start(befft[:, :], beff[:, :])
        fa0 = consts.tile([1, R, O], BF16)
        nc.scalar.dma_start(fa0[0:1, :, :], fa[:, 0, :])
        fv0 = consts.tile([1, R, O], BF16)
        nc.scalar.dma_start(fv0[0:1, :, :], fv[:, 0, :])

        acc = act_pool.tile([BT, O], F32, tag="acc", name="acc")
        for r in range(R):
            za = psum.tile([BT, 512], F32, tag="mm", name="za", bufs=3)
            zv = psum.tile([BT, 512], F32, tag="mm", name="zv", bufs=3)
            fat = wtile_load(fa[r], 1, KC, 0, O)
            fvt = wtile_load(fv[r], 1, KC, 0, O)
            for kc in range(KC):
                nc.tensor.matmul(za[:, :O], otherT[:, kc, :], fat[:, kc, :O],
                                 start=(kc == 0), stop=False)
            nc.tensor.matmul(za[:, :O], ones_t[0:1, :], fa0[0:1, r, :],
                             start=False, stop=True)
            for kc in range(KC):
                nc.tensor.matmul(zv[:, :O], ownT[:, kc, :], fvt[:, kc, :O],
                                 start=(kc == 0), stop=False)
            nc.tensor.matmul(zv[:, :O], ones_t[0:1, :], fv0[0:1, r, :],
                             start=False, stop=True)
            zasb = act_pool.tile([BT, O], F32, tag="zasb", name="zasb", bufs=2)
            nc.scalar.activation(zasb[:, :], za[:, :O], AF.Copy)
            t1 = act_pool.tile([BT, O], F32, tag="t1", name="t1", bufs=2)
            nc.vector.tensor_mul(t1[:, :], zasb[:, :], zv[:, :O])
            if r == 0:
                nc.vector.tensor_scalar_mul(acc[:, :], t1[:, :], fwt[:, 0:1])
            else:
                nc.vector.scalar_tensor_tensor(acc[:, :], t1[:, :],
                                               fwt[:, r:r + 1], acc[:, :],
                                               op0=ALU.mult, op1=ALU.add)

        fT = trans_lhsT(acc, O, "fT")
        psc = psum.tile([BT, 512], F32, tag="mm", name="psc", bufs=3)
        cwt = wtile_load(cls_w, 0, O // 128, 0, C)
        for kc in range(O // 128):
            nc.tensor.matmul(psc[:, :C], fT[:, kc, :], cwt[:, kc, :C],
                             start=(kc == 0), stop=(kc == O // 128 - 1))
        res = act_pool.tile([BT, C], F32, tag="res", name="res")
        nc.vector.tensor_add(res[:, :], psc[:, :C], befft[:, :])
        nc.sync.dma_start(out_ext[:, :], res[:, :])

    nc.compile()
    return nc


def _bf16(a):
    return np.ascontiguousarray(np.asarray(a, dtype=np.float32)).astype(
        ml_dtypes.bfloat16)


def kernel(x_path, x_omic, atsa_p, atsa_g, preeg_p, preeg_g, lmf, cls,
           _trace=False):
    if "nc" not in _BUILT:
        _BUILT["nc"] = _build_kernel()
    nc = _BUILT["nc"]

    def atsa_map(A):
        return {"a_w1": _bf16(A["w1"]), "a_b1": _bf16(A["b1"]).reshape(1, D),
                "a_wc": _bf16(A["wc"]), "a_bc": _bf16(A["bc"]).reshape(1, D),
                "a_w2": _bf16(A["w2"]), "a_b2": _bf16(A["b2"]).reshape(1, D)}

    def preeg_map(P):
        return {"p_wf1": _bf16(P["wf1"]),
                "p_bf1": _bf16(P["bf1"]).reshape(1, DH),
                "p_wf2": _bf16(P["wf2"]),
                "p_bf2": _bf16(P["bf2"]).reshape(1, D),
                "p_ew1": _bf16(P["ew1"]), "p_eb1": _bf16(P["eb1"]),
                "p_ew2": _bf16(P["ew2"]), "p_eb2": _bf16(P["eb2"])}

    fa_b, fv_b = _bf16(lmf["fa"]), _bf16(lmf["fv"])
    fw = np.asarray(lmf["fw"], np.float32).reshape(1, R)
    fwb = np.repeat(fw, BT, axis=0)
    fb = np.asarray(lmf["fb"], np.float32).reshape(1, O)
    cw = np.asarray(cls["w"], np.float32)
    cb = np.asarray(cls["b"], np.float32).reshape(1, C)
    beff = np.repeat(fb @ cw + cb, BT, axis=0).astype(np.float32)
    cwb = _bf16(cw)

    tower_p = {**atsa_map(atsa_p), **preeg_map(preeg_p),
               "lmf_fa": fa_b, "lmf_fv": fv_b}
    tower_g = {**atsa_map(atsa_g), **preeg_map(preeg_g),
               "lmf_fa": fv_b, "lmf_fv": fa_b}   # swapped on purpose
    common = {"lmf_fwb": fwb, "lmf_beff": beff, "cls_w": cwb}

    xp_b = _bf16(x_path).reshape(NCORES, BL, NP_, D)
    xo_b = _bf16(x_omic).reshape(NCORES, BL, NG, D)

    in_maps = []
    for s in range(NCORES):
        tower = tower_p if s < 4 else tower_g
        mp = dict(tower)
        mp.update(common)
        mp["xp"] = xp_b[s]
        mp["xo"] = xo_b[s]
        in_maps.append(mp)

    kernel.last_in_maps = in_maps
    res = run_bass_kernel_spmd(nc, in_maps, core_ids=list(range(NCORES)),
                               trace=_trace)
    out = np.concatenate([res.results[s]["out"] for s in range(4)], axis=0)
    if _trace:
        kernel.last_exec_time_ns = res.exec_time_ns
        kernel.last_results = res
    return out.astype(np.float32)
